# revision 20
# baseline (speedup 1.0000x reference)
"""Multi-head causal attention (B=8, S=1024, D=768, H=12) on 8 trn2 NeuronCores.

Strategy: data-parallel over batch (one batch element per core, no collectives).

Per-core dataflow (fp32r matmuls except A@V in bf16):
  - host passes x^T: Q^T/K^T via transposed projection (W stationary, x^T
    moving), V via natural projection (x^T stationary, W_v moving) -> no
    on-device transposes.
  - attention as S^T[k,q] = K @ Q^T per head; two heads (dh=64) packed into
    the 128-row PE array via row tiling, causal block-skip throughout.
  - softmax: exp on ScalarE straight out of PSUM ([128,1024] two-bank spans;
    1/8 scale folded into W_q host-side; no max-subtraction needed at these
    magnitudes); causal 0/1 bf16 mask multiply only on diagonal-crossing
    blocks; denominator free via a ones column appended to V (row 64 of the
    A@V PSUM); division folded into the PSUM->SBUF copy of A@V
    (fast reciprocal + DMA partition-broadcast through a DRAM scratch).
  - Q/K projection work-units are woven between attention tile groups so the
    PE fills exp-wait gaps and the HAM clock stays warm.
"""
import sys

if "/opt/trn_rl_repo" not in sys.path:
    sys.path.insert(0, "/opt/trn_rl_repo")

import numpy as np

B, S, D, H = 8, 1024, 768, 12
DH = 64
NC_ = 8
NT = D // 128    # 6
ST = S // 128    # 8
QC = S // 512    # 2
VPW = H * (DH + 1)  # 780

_compiled = None


def _build_masks():
    import ml_dtypes

    i = np.arange(128)[:, None, None]
    t = np.arange(4)[None, :, None]
    j = np.arange(512)[None, None, :]
    m = ((128 * t + i) <= j).astype(np.float32)
    return m.astype(ml_dtypes.bfloat16)


def _build_nc():
    import concourse.bass as bass
    import concourse.mybir as mybir
    import concourse.tile as tile
    from concourse import bacc

    F32 = mybir.dt.float32
    F32R = mybir.dt.float32r
    BF16 = mybir.dt.bfloat16
    AF = mybir.ActivationFunctionType
    MULT = mybir.AluOpType.mult

    nc = bacc.Bacc("TRN2", target_bir_lowering=False, debug=False)

    xT_d = nc.dram_tensor("xT", [D, S], F32, kind="ExternalInput")
    wq_d = nc.dram_tensor("wq", [D, D], F32, kind="ExternalInput")
    wk_d = nc.dram_tensor("wk", [D, D], F32, kind="ExternalInput")
    wv_d = nc.dram_tensor("wv", [D, D], F32, kind="ExternalInput")
    wp_d = nc.dram_tensor("wp", [D, D], F32, kind="ExternalInput")
    mask_d = nc.dram_tensor("masks", [128, 4, 512], BF16, kind="ExternalInput")
    y_d = nc.dram_tensor("y", [S, D], F32, kind="ExternalOutput")
    recip_d = nc.dram_tensor("recip_scratch", [H, QC, 512], F32)

    with tile.TileContext(nc) as tc:
        with (
            tc.tile_pool(name="static", bufs=1) as static,
            tc.tile_pool(name="w", bufs=12) as wpool,
            tc.tile_pool(name="pt", bufs=12) as ptpool,
            tc.tile_pool(name="small", bufs=2) as small,
            tc.tile_pool(name="rbp", bufs=3) as rbp,
            tc.tile_pool(name="y", bufs=2) as ypool,
            tc.tile_pool(name="psb", bufs=2, space="PSUM") as psb,
            tc.tile_pool(name="psproj", bufs=2, space="PSUM") as psproj,
            tc.tile_pool(name="pso", bufs=2, space="PSUM") as pso,
        ):
            # ---- persistent SBUF ----
            xT = static.tile([128, NT, S], F32R)
            qT = static.tile([128, NT, S], F32R)
            kT = static.tile([128, NT, S], F32R)
            vp = static.tile([128, ST, VPW], BF16)
            outT = static.tile([128, NT, S], F32R)
            msk = static.tile([128, 4, 512], BF16)

            for dc in range(NT):
                nc.sync.dma_start(xT[:, dc, :], xT_d[128 * dc:128 * (dc + 1), :].bitcast(F32R))
            nc.sync.dma_start(msk[:], mask_d[:])
            nc.vector.memset(vp[:], 1.0)

            # ---- stage C: v' = x @ W_v (natural layout) + ones cols ----
            wv_t = []
            for dc in range(NT):
                w = wpool.tile([128, D], F32R, tag="w")
                nc.sync.dma_start(w[:], wv_d[128 * dc:128 * (dc + 1), :].bitcast(F32R))
                wv_t.append(w)
            for st in range(ST):
                ps = psb.tile([128, 1024], F32, tag="big")
                for dc in range(NT):
                    nc.tensor.matmul(
                        ps[:, 0:512], xT[:, dc, 128 * st:128 * (st + 1)],
                        wv_t[dc][:, 0:512], start=(dc == 0), stop=(dc == NT - 1))
                for dc in range(NT):
                    nc.tensor.matmul(
                        ps[:, 512:768], xT[:, dc, 128 * st:128 * (st + 1)],
                        wv_t[dc][:, 512:768], start=(dc == 0), stop=(dc == NT - 1))
                dst = vp[:, st, :].rearrange("p (h e) -> p h e", e=DH + 1)
                nc.vector.tensor_copy(
                    out=dst[:, 0:8, 0:DH],
                    in_=ps[:, 0:512].rearrange("p (h d) -> p h d", d=DH))
                nc.vector.tensor_copy(
                    out=dst[:, 8:12, 0:DH],
                    in_=ps[:, 512:768].rearrange("p (h d) -> p h d", d=DH))

            # ---- Q^T/K^T projection work units (woven into attention) ----
            def emit_proj_unit(w_tiles, nt, dst, sc):
                ps = psproj.tile([128, 512], F32, tag="proj")
                for dc in range(NT):
                    nc.tensor.matmul(
                        ps[:],
                        w_tiles[dc][:, 128 * nt:128 * (nt + 1)],
                        xT[:, dc, 512 * sc:512 * (sc + 1)],
                        start=(dc == 0), stop=(dc == NT - 1))
                nc.vector.tensor_copy(out=dst[:, nt, 512 * sc:512 * (sc + 1)], in_=ps[:])

            proj_units = []

            def pop_unit():
                if proj_units:
                    proj_units.pop(0)[1]()

            def drain_units(hp_limit):
                while proj_units and proj_units[0][0] <= hp_limit:
                    proj_units.pop(0)[1]()

            wq_t, wk_t = [], []
            for dc in range(NT):
                w = wpool.tile([128, D], F32R, tag="w")
                nc.sync.dma_start(w[:], wq_d[128 * dc:128 * (dc + 1), :].bitcast(F32R))
                wq_t.append(w)
            for dc in range(NT):
                w = wpool.tile([128, D], F32R, tag="w")
                nc.sync.dma_start(w[:], wk_d[128 * dc:128 * (dc + 1), :].bitcast(F32R))
                wk_t.append(w)

            for hp in range(NT):
                for sc in range(2):
                    proj_units.append((hp, (lambda nt=hp, sc=sc: emit_proj_unit(wq_t, nt, qT, sc))))
                    proj_units.append((hp, (lambda nt=hp, sc=sc: emit_proj_unit(wk_t, nt, kT, sc))))

            # ---- attention blocks ----
            for hp in range(NT):
                drain_units(hp)

                for qc in range(QC):
                    K = 4 * (qc + 1)
                    pts = {0: [], 1: []}
                    for kp in range(K // 2):
                        tiles = {}
                        for hh in range(2):
                            t_ = psb.tile([128, 1024], F32, tag="big", name=f"st_{hp}_{qc}_{kp}_{hh}")
                            tiles[hh] = t_
                        for j in range(2):
                            kc = 2 * kp + j
                            for hh in range(2):
                                rows = slice(64 * hh, 64 * (hh + 1))
                                nc.tensor.matmul(
                                    tiles[hh][:, 512 * j:512 * (j + 1)],
                                    kT[rows, hp, 128 * kc:128 * (kc + 1)],
                                    qT[rows, hp, 512 * qc:512 * (qc + 1)],
                                    start=True, stop=True,
                                    tile_position=(64 * hh, 0))
                        for hh in range(2):
                            pt = ptpool.tile([128, 1024], BF16, tag="pt")
                            nc.scalar.activation(pt[:], tiles[hh][:], AF.Exp)
                            for j in range(2):
                                t = 2 * kp + j - 4 * qc
                                if 0 <= t <= 3:
                                    half = pt[:, 512 * j:512 * (j + 1)]
                                    nc.vector.tensor_tensor(half, half, msk[:, t, :], MULT)
                            pts[hh].append(pt)
                        pop_unit()

                    for hh in range(2):
                        h = 2 * hp + hh
                        rows = slice(64 * hh, 64 * (hh + 1))
                        po = pso.tile([65, 512], F32, tag="po")
                        for kc in range(K):
                            nc.tensor.matmul(
                                po[:],
                                vp[:, kc, 65 * h:65 * (h + 1)],
                                pts[hh][kc // 2][:, 512 * (kc % 2):512 * (kc % 2 + 1)],
                                start=(kc == 0), stop=(kc == K - 1))
                        den = small.tile([1, 512], F32, tag="den")
                        nc.vector.tensor_copy(out=den[:], in_=po[64:65, :])
                        rc = small.tile([1, 512], F32, tag="rc")
                        nc.vector.reciprocal_approx_fast(out=rc[:], in_=den[:])
                        nc.gpsimd.dma_start(recip_d[h, qc, :], rc[:])
                        rb = rbp.tile([64, 512], F32, tag="rb")
                        sl = recip_d[h, qc, :]
                        bc_ap = bass.AP(tensor=sl.tensor, offset=sl.offset,
                                        ap=[[0, 64]] + list(sl.ap))
                        nc.gpsimd.dma_start(rb[:], bc_ap)
                        nc.vector.tensor_tensor(
                            outT[rows, hp, 512 * qc:512 * (qc + 1)],
                            po[0:64, :], rb[:], MULT)

            # ---- stage E: y = out @ W_proj ----
            wp_t = []
            for dc in range(NT):
                w = wpool.tile([128, D], F32R, tag="w")
                nc.sync.dma_start(w[:], wp_d[128 * dc:128 * (dc + 1), :].bitcast(F32R))
                wp_t.append(w)
            for st in range(ST):
                ps = psb.tile([128, 1024], F32, tag="big")
                for dc in range(NT):
                    nc.tensor.matmul(
                        ps[:, 0:512], outT[:, dc, 128 * st:128 * (st + 1)],
                        wp_t[dc][:, 0:512], start=(dc == 0), stop=(dc == NT - 1))
                for dc in range(NT):
                    nc.tensor.matmul(
                        ps[:, 512:768], outT[:, dc, 128 * st:128 * (st + 1)],
                        wp_t[dc][:, 512:768], start=(dc == 0), stop=(dc == NT - 1))
                y_sb = ypool.tile([128, D], F32, tag="y")
                nc.vector.tensor_copy(out=y_sb[:], in_=ps[:, 0:768])
                nc.sync.dma_start(y_d[128 * st:128 * (st + 1), :], y_sb[:])

    nc.compile()
    return nc


def _get_compiled():
    global _compiled
    if _compiled is None:
        _compiled = _build_nc()
    return _compiled


def kernel(x, W_attn, W_proj):
    from concourse.bass_utils import run_bass_kernel_spmd

    x = np.asarray(x, dtype=np.float32)
    W_attn = np.asarray(W_attn, dtype=np.float32)
    W_proj = np.asarray(W_proj, dtype=np.float32)

    xT = np.ascontiguousarray(np.transpose(x, (0, 2, 1)))
    wq = np.ascontiguousarray(W_attn[:, 0:D]) * np.float32(0.125)
    wk = np.ascontiguousarray(W_attn[:, D:2 * D])
    wv = np.ascontiguousarray(W_attn[:, 2 * D:3 * D])
    masks = _build_masks()

    nc = _get_compiled()
    in_maps = [
        {"xT": xT[b], "wq": wq, "wk": wk, "wv": wv, "wp": W_proj, "masks": masks}
        for b in range(B)
    ]
    res = run_bass_kernel_spmd(nc, in_maps, list(range(NC_)))
    y = np.stack([res.results[b]["y"] for b in range(B)], axis=0)
    return y.astype(np.float32)


# revision 22
# speedup vs baseline: 1.1814x; 1.1814x over previous
"""Multi-head causal attention (B=8, S=1024, D=768, H=12) on 8 trn2 NeuronCores.

Strategy: data-parallel over batch (one batch element per core, no collectives).

Per-core dataflow (fp32r matmuls except A@V in bf16):
  - host passes x^T: Q^T/K^T via transposed projection (W stationary, x^T
    moving), V via natural projection (x^T stationary, W_v moving) -> no
    on-device transposes.
  - attention as S^T[k,q] = K @ Q^T per head; two heads (dh=64) packed into
    the 128-row PE array via row tiling, causal block-skip throughout.
  - softmax: exp on ScalarE straight out of PSUM ([128,1024] two-bank spans;
    1/8 scale folded into W_q host-side; no max-subtraction needed at these
    magnitudes); causal 0/1 bf16 mask multiply only on diagonal-crossing
    blocks; denominator free via a ones column appended to V (row 64 of the
    A@V PSUM); division folded into the PSUM->SBUF copy of A@V
    (fast reciprocal + DMA partition-broadcast through a DRAM scratch).
  - Q/K projection work-units are woven between attention tile groups so the
    PE fills exp-wait gaps and the HAM clock stays warm.
"""
import sys

if "/opt/trn_rl_repo" not in sys.path:
    sys.path.insert(0, "/opt/trn_rl_repo")

import numpy as np

B, S, D, H = 8, 1024, 768, 12
DH = 64
NC_ = 8
NT = D // 128    # 6
ST = S // 128    # 8
QC = S // 512    # 2
VPW = H * (DH + 1)  # 780

_compiled = None


def _build_masks():
    import ml_dtypes

    i = np.arange(128)[:, None, None]
    t = np.arange(4)[None, :, None]
    j = np.arange(512)[None, None, :]
    m = ((128 * t + i) <= j).astype(np.float32)
    return m.astype(ml_dtypes.bfloat16)


def _build_nc():
    import concourse.bass as bass
    import concourse.mybir as mybir
    import concourse.tile as tile
    from concourse import bacc

    F32 = mybir.dt.float32
    F32R = mybir.dt.float32r
    BF16 = mybir.dt.bfloat16
    AF = mybir.ActivationFunctionType
    MULT = mybir.AluOpType.mult

    nc = bacc.Bacc("TRN2", target_bir_lowering=False, debug=False)

    xT_d = nc.dram_tensor("xT", [D, S], F32, kind="ExternalInput")
    wq_d = nc.dram_tensor("wq", [D, D], F32, kind="ExternalInput")
    wk_d = nc.dram_tensor("wk", [D, D], F32, kind="ExternalInput")
    wv_d = nc.dram_tensor("wv", [D, D], F32, kind="ExternalInput")
    wp_d = nc.dram_tensor("wp", [D, D], F32, kind="ExternalInput")
    mask_d = nc.dram_tensor("masks", [128, 4, 512], BF16, kind="ExternalInput")
    y_d = nc.dram_tensor("y", [S, D], F32, kind="ExternalOutput")
    recip_d = nc.dram_tensor("recip_scratch", [H, QC, 512], F32)

    with tile.TileContext(nc) as tc:
        with (
            tc.tile_pool(name="static", bufs=1) as static,
            tc.tile_pool(name="w", bufs=12) as wpool,
            tc.tile_pool(name="pt", bufs=13) as ptpool,
            tc.tile_pool(name="small", bufs=2) as small,
            tc.tile_pool(name="rbp", bufs=2) as rbp,
            tc.tile_pool(name="y", bufs=2) as ypool,
            tc.tile_pool(name="psb", bufs=2, space="PSUM") as psb,
            tc.tile_pool(name="psproj", bufs=2, space="PSUM") as psproj,
            tc.tile_pool(name="pso", bufs=2, space="PSUM") as pso,
        ):
            # ---- persistent SBUF ----
            xT = static.tile([128, NT, S], F32R)
            qT = static.tile([128, NT, S], F32R)
            kT = static.tile([128, NT, S], F32R)
            vp = static.tile([128, ST, VPW], BF16)
            outT = static.tile([128, NT, S], F32R)
            msk = static.tile([128, 4, 512], BF16)

            for dc in range(NT):
                nc.sync.dma_start(xT[:, dc, :], xT_d[128 * dc:128 * (dc + 1), :].bitcast(F32R))
            nc.sync.dma_start(msk[:], mask_d[:])
            nc.vector.memset(vp[:], 1.0)

            # ---- stage C: v' = x @ W_v (natural layout) + ones cols ----
            wv_t = []
            for dc in range(NT):
                w = wpool.tile([128, D], F32R, tag="w")
                nc.sync.dma_start(w[:], wv_d[128 * dc:128 * (dc + 1), :].bitcast(F32R))
                wv_t.append(w)
            for st in range(ST):
                ps = psb.tile([128, 1024], F32, tag="big")
                for dc in range(NT):
                    nc.tensor.matmul(
                        ps[:, 0:512], xT[:, dc, 128 * st:128 * (st + 1)],
                        wv_t[dc][:, 0:512], start=(dc == 0), stop=(dc == NT - 1))
                for dc in range(NT):
                    nc.tensor.matmul(
                        ps[:, 512:768], xT[:, dc, 128 * st:128 * (st + 1)],
                        wv_t[dc][:, 512:768], start=(dc == 0), stop=(dc == NT - 1))
                dst = vp[:, st, :].rearrange("p (h e) -> p h e", e=DH + 1)
                nc.vector.tensor_copy(
                    out=dst[:, 0:8, 0:DH],
                    in_=ps[:, 0:512].rearrange("p (h d) -> p h d", d=DH))
                nc.vector.tensor_copy(
                    out=dst[:, 8:12, 0:DH],
                    in_=ps[:, 512:768].rearrange("p (h d) -> p h d", d=DH))

            # ---- Q^T/K^T projection work units (woven into attention) ----
            def emit_proj_unit(w_tiles, nt, dst, sc):
                ps = psproj.tile([128, 512], F32, tag="proj")
                for dc in range(NT):
                    nc.tensor.matmul(
                        ps[:],
                        w_tiles[dc][:, 128 * nt:128 * (nt + 1)],
                        xT[:, dc, 512 * sc:512 * (sc + 1)],
                        start=(dc == 0), stop=(dc == NT - 1))
                nc.any.tensor_copy(out=dst[:, nt, 512 * sc:512 * (sc + 1)], in_=ps[:])

            proj_units = []

            def pop_unit():
                if proj_units:
                    proj_units.pop(0)[1]()

            def drain_units(hp_limit):
                while proj_units and proj_units[0][0] <= hp_limit:
                    proj_units.pop(0)[1]()

            wq_t, wk_t = [], []
            for dc in range(NT):
                w = wpool.tile([128, D], F32R, tag="w")
                nc.sync.dma_start(w[:], wq_d[128 * dc:128 * (dc + 1), :].bitcast(F32R))
                wq_t.append(w)
            for dc in range(NT):
                w = wpool.tile([128, D], F32R, tag="w")
                nc.sync.dma_start(w[:], wk_d[128 * dc:128 * (dc + 1), :].bitcast(F32R))
                wk_t.append(w)

            for hp in range(NT):
                for sc in range(2):
                    proj_units.append((hp, (lambda nt=hp, sc=sc: emit_proj_unit(wq_t, nt, qT, sc))))
                    proj_units.append((hp, (lambda nt=hp, sc=sc: emit_proj_unit(wk_t, nt, kT, sc))))

            # ---- attention blocks ----
            for hp in range(NT):
                drain_units(hp)

                for qc in range(QC):
                    K = 4 * (qc + 1)
                    pts = {0: [], 1: []}
                    for kp in range(K // 2):
                        tiles = {}
                        for hh in range(2):
                            t_ = psb.tile([128, 1024], F32, tag="big", name=f"st_{hp}_{qc}_{kp}_{hh}")
                            tiles[hh] = t_
                        for j in range(2):
                            kc = 2 * kp + j
                            for hh in range(2):
                                rows = slice(64 * hh, 64 * (hh + 1))
                                nc.tensor.matmul(
                                    tiles[hh][:, 512 * j:512 * (j + 1)],
                                    kT[rows, hp, 128 * kc:128 * (kc + 1)],
                                    qT[rows, hp, 512 * qc:512 * (qc + 1)],
                                    start=True, stop=True,
                                    tile_position=(64 * hh, 0))
                        for hh in range(2):
                            pt = ptpool.tile([128, 1024], BF16, tag="pt")
                            nc.scalar.activation(pt[:], tiles[hh][:], AF.Exp)
                            for j in range(2):
                                t = 2 * kp + j - 4 * qc
                                if 0 <= t <= 3:
                                    half = pt[:, 512 * j:512 * (j + 1)]
                                    nc.vector.tensor_tensor(half, half, msk[:, t, :], MULT)
                            pts[hh].append(pt)
                        pop_unit()

                    for hh in range(2):
                        h = 2 * hp + hh
                        rows = slice(64 * hh, 64 * (hh + 1))
                        po = pso.tile([65, 512], F32, tag="po")
                        for kc in range(K):
                            nc.tensor.matmul(
                                po[:],
                                vp[:, kc, 65 * h:65 * (h + 1)],
                                pts[hh][kc // 2][:, 512 * (kc % 2):512 * (kc % 2 + 1)],
                                start=(kc == 0), stop=(kc == K - 1))
                        den = small.tile([1, 512], F32, tag="den")
                        nc.vector.tensor_copy(out=den[:], in_=po[64:65, :])
                        rc = small.tile([1, 512], F32, tag="rc")
                        nc.vector.reciprocal_approx_fast(out=rc[:], in_=den[:])
                        nc.sync.dma_start(recip_d[h, qc, :], rc[:])
                        rb = rbp.tile([64, 512], F32, tag="rb")
                        sl = recip_d[h, qc, :]
                        bc_ap = bass.AP(tensor=sl.tensor, offset=sl.offset,
                                        ap=[[0, 64]] + list(sl.ap))
                        nc.sync.dma_start(rb[:], bc_ap)
                        nc.vector.tensor_tensor(
                            outT[rows, hp, 512 * qc:512 * (qc + 1)],
                            po[0:64, :], rb[:], MULT)

            # ---- stage E: y = out @ W_proj ----
            wp_t = []
            for dc in range(NT):
                w = wpool.tile([128, D], F32R, tag="w")
                nc.sync.dma_start(w[:], wp_d[128 * dc:128 * (dc + 1), :].bitcast(F32R))
                wp_t.append(w)
            for st in range(ST):
                ps = psb.tile([128, 1024], F32, tag="big")
                for dc in range(NT):
                    nc.tensor.matmul(
                        ps[:, 0:512], outT[:, dc, 128 * st:128 * (st + 1)],
                        wp_t[dc][:, 0:512], start=(dc == 0), stop=(dc == NT - 1))
                for dc in range(NT):
                    nc.tensor.matmul(
                        ps[:, 512:768], outT[:, dc, 128 * st:128 * (st + 1)],
                        wp_t[dc][:, 512:768], start=(dc == 0), stop=(dc == NT - 1))
                y_sb = ypool.tile([128, D], F32, tag="y")
                nc.any.tensor_copy(out=y_sb[:], in_=ps[:, 0:768])
                nc.sync.dma_start(y_d[128 * st:128 * (st + 1), :], y_sb[:])

    nc.compile()
    return nc


def _get_compiled():
    global _compiled
    if _compiled is None:
        _compiled = _build_nc()
    return _compiled


def kernel(x, W_attn, W_proj):
    from concourse.bass_utils import run_bass_kernel_spmd

    x = np.asarray(x, dtype=np.float32)
    W_attn = np.asarray(W_attn, dtype=np.float32)
    W_proj = np.asarray(W_proj, dtype=np.float32)

    xT = np.ascontiguousarray(np.transpose(x, (0, 2, 1)))
    wq = np.ascontiguousarray(W_attn[:, 0:D]) * np.float32(0.125)
    wk = np.ascontiguousarray(W_attn[:, D:2 * D])
    wv = np.ascontiguousarray(W_attn[:, 2 * D:3 * D])
    masks = _build_masks()

    nc = _get_compiled()
    in_maps = [
        {"xT": xT[b], "wq": wq, "wk": wk, "wv": wv, "wp": W_proj, "masks": masks}
        for b in range(B)
    ]
    res = run_bass_kernel_spmd(nc, in_maps, list(range(NC_)))
    y = np.stack([res.results[b]["y"] for b in range(B)], axis=0)
    return y.astype(np.float32)


# revision 23
# speedup vs baseline: 1.1862x; 1.0040x over previous
"""Multi-head causal attention (B=8, S=1024, D=768, H=12) on 8 trn2 NeuronCores.

Strategy: data-parallel over batch (one batch element per core, no collectives).

Per-core dataflow (fp32r matmuls except A@V in bf16):
  - host passes x^T: Q^T/K^T via transposed projection (W stationary, x^T
    moving), V via natural projection (x^T stationary, W_v moving) -> no
    on-device transposes.
  - attention as S^T[k,q] = K @ Q^T per head; two heads (dh=64) packed into
    the 128-row PE array via row tiling, causal block-skip throughout.
  - softmax: exp on ScalarE straight out of PSUM ([128,1024] two-bank spans;
    1/8 scale folded into W_q host-side; no max-subtraction needed at these
    magnitudes); causal 0/1 bf16 mask multiply only on diagonal-crossing
    blocks; denominator free via a ones column appended to V (row 64 of the
    A@V PSUM); division folded into the PSUM->SBUF copy of A@V
    (fast reciprocal + DMA partition-broadcast through a DRAM scratch).
  - Q/K projection work-units are woven between attention tile groups so the
    PE fills exp-wait gaps and the HAM clock stays warm.
"""
import sys

if "/opt/trn_rl_repo" not in sys.path:
    sys.path.insert(0, "/opt/trn_rl_repo")

import numpy as np

B, S, D, H = 8, 1024, 768, 12
DH = 64
NC_ = 8
NT = D // 128    # 6
ST = S // 128    # 8
QC = S // 512    # 2
VPW = H * (DH + 1)  # 780

_compiled = None


def _build_masks():
    import ml_dtypes

    i = np.arange(128)[:, None, None]
    t = np.arange(4)[None, :, None]
    j = np.arange(512)[None, None, :]
    m = ((128 * t + i) <= j).astype(np.float32)
    return m.astype(ml_dtypes.bfloat16)


def _build_nc():
    import concourse.bass as bass
    import concourse.mybir as mybir
    import concourse.tile as tile
    from concourse import bacc

    F32 = mybir.dt.float32
    F32R = mybir.dt.float32r
    BF16 = mybir.dt.bfloat16
    AF = mybir.ActivationFunctionType
    MULT = mybir.AluOpType.mult

    nc = bacc.Bacc("TRN2", target_bir_lowering=False, debug=False)

    xT_d = nc.dram_tensor("xT", [D, S], F32, kind="ExternalInput")
    wq_d = nc.dram_tensor("wq", [D, D], F32, kind="ExternalInput")
    wk_d = nc.dram_tensor("wk", [D, D], F32, kind="ExternalInput")
    wv_d = nc.dram_tensor("wv", [D, D], F32, kind="ExternalInput")
    wp_d = nc.dram_tensor("wp", [D, D], F32, kind="ExternalInput")
    mask_d = nc.dram_tensor("masks", [128, 4, 512], BF16, kind="ExternalInput")
    y_d = nc.dram_tensor("y", [S, D], F32, kind="ExternalOutput")
    recip_d = nc.dram_tensor("recip_scratch", [H, QC, 512], F32)

    with tile.TileContext(nc) as tc:
        with (
            tc.tile_pool(name="static", bufs=1) as static,
            tc.tile_pool(name="w", bufs=12) as wpool,
            tc.tile_pool(name="pt", bufs=11) as ptpool,
            tc.tile_pool(name="small", bufs=2) as small,
            tc.tile_pool(name="rbp", bufs=2) as rbp,
            tc.tile_pool(name="mk", bufs=10) as mkpool,
            tc.tile_pool(name="y", bufs=2) as ypool,
            tc.tile_pool(name="psb", bufs=2, space="PSUM") as psb,
            tc.tile_pool(name="psproj", bufs=2, space="PSUM") as psproj,
            tc.tile_pool(name="pso", bufs=2, space="PSUM") as pso,
        ):
            # ---- persistent SBUF ----
            xT = static.tile([128, NT, S], F32R)
            qT = static.tile([128, NT, S], F32R)
            kT = static.tile([128, NT, S], F32R)
            vp = static.tile([128, ST, VPW], BF16)
            outT = static.tile([128, NT, S], F32R)
            msk = static.tile([128, 4, 512], BF16)

            for dc in range(NT):
                nc.sync.dma_start(xT[:, dc, :], xT_d[128 * dc:128 * (dc + 1), :].bitcast(F32R))
            nc.sync.dma_start(msk[:], mask_d[:])
            nc.vector.memset(vp[:], 1.0)

            # ---- stage C: v' = x @ W_v (natural layout) + ones cols ----
            wv_t = []
            for dc in range(NT):
                w = wpool.tile([128, D], F32R, tag="w")
                nc.sync.dma_start(w[:], wv_d[128 * dc:128 * (dc + 1), :].bitcast(F32R))
                wv_t.append(w)
            for st in range(ST):
                ps = psb.tile([128, 1024], F32, tag="big")
                for dc in range(NT):
                    nc.tensor.matmul(
                        ps[:, 0:512], xT[:, dc, 128 * st:128 * (st + 1)],
                        wv_t[dc][:, 0:512], start=(dc == 0), stop=(dc == NT - 1))
                for dc in range(NT):
                    nc.tensor.matmul(
                        ps[:, 512:768], xT[:, dc, 128 * st:128 * (st + 1)],
                        wv_t[dc][:, 512:768], start=(dc == 0), stop=(dc == NT - 1))
                dst = vp[:, st, :].rearrange("p (h e) -> p h e", e=DH + 1)
                nc.vector.tensor_copy(
                    out=dst[:, 0:8, 0:DH],
                    in_=ps[:, 0:512].rearrange("p (h d) -> p h d", d=DH))
                nc.vector.tensor_copy(
                    out=dst[:, 8:12, 0:DH],
                    in_=ps[:, 512:768].rearrange("p (h d) -> p h d", d=DH))

            # ---- Q^T/K^T projection work units (woven into attention) ----
            def emit_proj_unit(w_tiles, nt, dst, sc):
                ps = psproj.tile([128, 512], F32, tag="proj")
                for dc in range(NT):
                    nc.tensor.matmul(
                        ps[:],
                        w_tiles[dc][:, 128 * nt:128 * (nt + 1)],
                        xT[:, dc, 512 * sc:512 * (sc + 1)],
                        start=(dc == 0), stop=(dc == NT - 1))
                nc.any.tensor_copy(out=dst[:, nt, 512 * sc:512 * (sc + 1)], in_=ps[:])

            proj_units = []

            def pop_unit():
                if proj_units:
                    proj_units.pop(0)[1]()

            def drain_units(hp_limit):
                while proj_units and proj_units[0][0] <= hp_limit:
                    proj_units.pop(0)[1]()

            wq_t, wk_t = [], []
            for dc in range(NT):
                w = wpool.tile([128, D], F32R, tag="w")
                nc.sync.dma_start(w[:], wq_d[128 * dc:128 * (dc + 1), :].bitcast(F32R))
                wq_t.append(w)
            for dc in range(NT):
                w = wpool.tile([128, D], F32R, tag="w")
                nc.sync.dma_start(w[:], wk_d[128 * dc:128 * (dc + 1), :].bitcast(F32R))
                wk_t.append(w)

            for hp in range(NT):
                for sc in range(2):
                    proj_units.append((hp, (lambda nt=hp, sc=sc: emit_proj_unit(wq_t, nt, qT, sc))))
                    proj_units.append((hp, (lambda nt=hp, sc=sc: emit_proj_unit(wk_t, nt, kT, sc))))

            # ---- attention blocks ----
            for hp in range(NT):
                drain_units(hp)

                for qc in range(QC):
                    K = 4 * (qc + 1)
                    pts = {0: [], 1: []}
                    for kp in range(K // 2):
                        tiles = {}
                        for hh in range(2):
                            t_ = psb.tile([128, 1024], F32, tag="big", name=f"st_{hp}_{qc}_{kp}_{hh}")
                            tiles[hh] = t_
                        for j in range(2):
                            kc = 2 * kp + j
                            for hh in range(2):
                                rows = slice(64 * hh, 64 * (hh + 1))
                                nc.tensor.matmul(
                                    tiles[hh][:, 512 * j:512 * (j + 1)],
                                    kT[rows, hp, 128 * kc:128 * (kc + 1)],
                                    qT[rows, hp, 512 * qc:512 * (qc + 1)],
                                    start=True, stop=True,
                                    tile_position=(64 * hh, 0))
                        for hh in range(2):
                            pt = ptpool.tile([128, 1024], BF16, tag="pt")
                            nc.scalar.activation(pt[:], tiles[hh][:], AF.Exp)
                            halves = []
                            for j in range(2):
                                t = 2 * kp + j - 4 * qc
                                src_half = pt[:, 512 * j:512 * (j + 1)]
                                if 0 <= t <= 3:
                                    mh = mkpool.tile([128, 512], BF16, tag="mh")
                                    nc.vector.tensor_tensor(mh[:], src_half, msk[:, t, :], MULT)
                                    halves.append(mh[:])
                                else:
                                    halves.append(src_half)
                            pts[hh].append(halves)
                        pop_unit()

                    for hh in range(2):
                        h = 2 * hp + hh
                        rows = slice(64 * hh, 64 * (hh + 1))
                        po = pso.tile([65, 512], F32, tag="po")
                        for kc in range(K):
                            nc.tensor.matmul(
                                po[:],
                                vp[:, kc, 65 * h:65 * (h + 1)],
                                pts[hh][kc // 2][kc % 2],
                                start=(kc == 0), stop=(kc == K - 1))
                        den = small.tile([1, 512], F32, tag="den")
                        nc.vector.tensor_copy(out=den[:], in_=po[64:65, :])
                        rc = small.tile([1, 512], F32, tag="rc")
                        nc.vector.reciprocal_approx_fast(out=rc[:], in_=den[:])
                        nc.sync.dma_start(recip_d[h, qc, :], rc[:])
                        rb = rbp.tile([64, 512], F32, tag="rb")
                        sl = recip_d[h, qc, :]
                        bc_ap = bass.AP(tensor=sl.tensor, offset=sl.offset,
                                        ap=[[0, 64]] + list(sl.ap))
                        nc.sync.dma_start(rb[:], bc_ap)
                        nc.vector.tensor_tensor(
                            outT[rows, hp, 512 * qc:512 * (qc + 1)],
                            po[0:64, :], rb[:], MULT)

            # ---- stage E: y = out @ W_proj ----
            wp_t = []
            for dc in range(NT):
                w = wpool.tile([128, D], F32R, tag="w")
                nc.sync.dma_start(w[:], wp_d[128 * dc:128 * (dc + 1), :].bitcast(F32R))
                wp_t.append(w)
            for st in range(ST):
                ps = psb.tile([128, 1024], F32, tag="big")
                for dc in range(NT):
                    nc.tensor.matmul(
                        ps[:, 0:512], outT[:, dc, 128 * st:128 * (st + 1)],
                        wp_t[dc][:, 0:512], start=(dc == 0), stop=(dc == NT - 1))
                for dc in range(NT):
                    nc.tensor.matmul(
                        ps[:, 512:768], outT[:, dc, 128 * st:128 * (st + 1)],
                        wp_t[dc][:, 512:768], start=(dc == 0), stop=(dc == NT - 1))
                y_sb = ypool.tile([128, D], F32, tag="y")
                nc.any.tensor_copy(out=y_sb[:], in_=ps[:, 0:768])
                nc.sync.dma_start(y_d[128 * st:128 * (st + 1), :], y_sb[:])

    nc.compile()
    return nc


def _get_compiled():
    global _compiled
    if _compiled is None:
        _compiled = _build_nc()
    return _compiled


def kernel(x, W_attn, W_proj):
    from concourse.bass_utils import run_bass_kernel_spmd

    x = np.asarray(x, dtype=np.float32)
    W_attn = np.asarray(W_attn, dtype=np.float32)
    W_proj = np.asarray(W_proj, dtype=np.float32)

    xT = np.ascontiguousarray(np.transpose(x, (0, 2, 1)))
    wq = np.ascontiguousarray(W_attn[:, 0:D]) * np.float32(0.125)
    wk = np.ascontiguousarray(W_attn[:, D:2 * D])
    wv = np.ascontiguousarray(W_attn[:, 2 * D:3 * D])
    masks = _build_masks()

    nc = _get_compiled()
    in_maps = [
        {"xT": xT[b], "wq": wq, "wk": wk, "wv": wv, "wp": W_proj, "masks": masks}
        for b in range(B)
    ]
    res = run_bass_kernel_spmd(nc, in_maps, list(range(NC_)))
    y = np.stack([res.results[b]["y"] for b in range(B)], axis=0)
    return y.astype(np.float32)


# revision 24
# speedup vs baseline: 1.2338x; 1.0401x over previous
"""Multi-head causal attention (B=8, S=1024, D=768, H=12) on 8 trn2 NeuronCores.

Strategy: data-parallel over batch (one batch element per core, no collectives).

Per-core dataflow (fp32r matmuls except A@V in bf16):
  - host passes x^T: Q^T/K^T via transposed projection (W stationary, x^T
    moving), V via natural projection (x^T stationary, W_v moving) -> no
    on-device transposes.
  - attention as S^T[k,q] = K @ Q^T per head; two heads (dh=64) packed into
    the 128-row PE array via row tiling, causal block-skip throughout.
  - softmax: exp on ScalarE straight out of PSUM ([128,1024] two-bank spans;
    1/8 scale folded into W_q host-side; no max-subtraction needed at these
    magnitudes); causal 0/1 bf16 mask multiply only on diagonal-crossing
    blocks; denominator free via a ones column appended to V (row 64 of the
    A@V PSUM); division folded into the PSUM->SBUF copy of A@V
    (fast reciprocal + DMA partition-broadcast through a DRAM scratch).
  - Q/K projection work-units are woven between attention tile groups so the
    PE fills exp-wait gaps and the HAM clock stays warm.
"""
import sys

if "/opt/trn_rl_repo" not in sys.path:
    sys.path.insert(0, "/opt/trn_rl_repo")

import numpy as np

B, S, D, H = 8, 1024, 768, 12
DH = 64
NC_ = 8
NT = D // 128    # 6
ST = S // 128    # 8
QC = S // 512    # 2
VPW = H * (DH + 1)  # 780

_compiled = None


def _build_masks():
    import ml_dtypes

    i = np.arange(128)[:, None, None]
    t = np.arange(4)[None, :, None]
    j = np.arange(512)[None, None, :]
    m = ((128 * t + i) <= j).astype(np.float32)
    return m.astype(ml_dtypes.bfloat16)


def _build_nc():
    import concourse.bass as bass
    import concourse.mybir as mybir
    import concourse.tile as tile
    from concourse import bacc

    F32 = mybir.dt.float32
    F32R = mybir.dt.float32r
    BF16 = mybir.dt.bfloat16
    AF = mybir.ActivationFunctionType
    MULT = mybir.AluOpType.mult

    nc = bacc.Bacc("TRN2", target_bir_lowering=False, debug=False)

    xT_d = nc.dram_tensor("xT", [D, S], F32, kind="ExternalInput")
    wq_d = nc.dram_tensor("wq", [D, D], F32, kind="ExternalInput")
    wk_d = nc.dram_tensor("wk", [D, D], F32, kind="ExternalInput")
    wv_d = nc.dram_tensor("wv", [D, D], F32, kind="ExternalInput")
    wp_d = nc.dram_tensor("wp", [D, D], F32, kind="ExternalInput")
    mask_d = nc.dram_tensor("masks", [128, 4, 512], BF16, kind="ExternalInput")
    y_d = nc.dram_tensor("y", [S, D], F32, kind="ExternalOutput")
    recip_d = nc.dram_tensor("recip_scratch", [H, QC, 512], F32)

    with tile.TileContext(nc) as tc:
        with (
            tc.tile_pool(name="static", bufs=1) as static,
            tc.tile_pool(name="w", bufs=12) as wpool,
            tc.tile_pool(name="pt", bufs=11) as ptpool,
            tc.tile_pool(name="small", bufs=2) as small,
            tc.tile_pool(name="rbp", bufs=2) as rbp,
            tc.tile_pool(name="mk", bufs=10) as mkpool,
            tc.tile_pool(name="y", bufs=2) as ypool,
            tc.tile_pool(name="psb", bufs=2, space="PSUM") as psb,
            tc.tile_pool(name="psproj", bufs=1, space="PSUM") as psproj,
            tc.tile_pool(name="pso", bufs=3, space="PSUM") as pso,
        ):
            # ---- persistent SBUF ----
            xT = static.tile([128, NT, S], F32R)
            qT = static.tile([128, NT, S], F32R)
            kT = static.tile([128, NT, S], F32R)
            vp = static.tile([128, ST, VPW], BF16)
            outT = static.tile([128, NT, S], F32R)
            msk = static.tile([128, 4, 512], BF16)

            for dc in range(NT):
                nc.sync.dma_start(xT[:, dc, :], xT_d[128 * dc:128 * (dc + 1), :].bitcast(F32R))
            nc.sync.dma_start(msk[:], mask_d[:])
            nc.vector.memset(vp[:], 1.0)

            # ---- stage C: v' = x @ W_v (natural layout) + ones cols ----
            wv_t = []
            for dc in range(NT):
                w = wpool.tile([128, D], F32R, tag="w")
                nc.sync.dma_start(w[:], wv_d[128 * dc:128 * (dc + 1), :].bitcast(F32R))
                wv_t.append(w)
            for st in range(ST):
                ps = psb.tile([128, 1024], F32, tag="big")
                for dc in range(NT):
                    nc.tensor.matmul(
                        ps[:, 0:512], xT[:, dc, 128 * st:128 * (st + 1)],
                        wv_t[dc][:, 0:512], start=(dc == 0), stop=(dc == NT - 1))
                for dc in range(NT):
                    nc.tensor.matmul(
                        ps[:, 512:768], xT[:, dc, 128 * st:128 * (st + 1)],
                        wv_t[dc][:, 512:768], start=(dc == 0), stop=(dc == NT - 1))
                dst = vp[:, st, :].rearrange("p (h e) -> p h e", e=DH + 1)
                nc.vector.tensor_copy(
                    out=dst[:, 0:8, 0:DH],
                    in_=ps[:, 0:512].rearrange("p (h d) -> p h d", d=DH))
                nc.vector.tensor_copy(
                    out=dst[:, 8:12, 0:DH],
                    in_=ps[:, 512:768].rearrange("p (h d) -> p h d", d=DH))

            # ---- Q^T/K^T projection work units (woven into attention) ----
            def emit_proj_unit(w_tiles, nt, dst, sc):
                ps = psproj.tile([128, 512], F32, tag="proj")
                for dc in range(NT):
                    nc.tensor.matmul(
                        ps[:],
                        w_tiles[dc][:, 128 * nt:128 * (nt + 1)],
                        xT[:, dc, 512 * sc:512 * (sc + 1)],
                        start=(dc == 0), stop=(dc == NT - 1))
                nc.any.tensor_copy(out=dst[:, nt, 512 * sc:512 * (sc + 1)], in_=ps[:])

            proj_units = []

            def pop_unit():
                if proj_units:
                    proj_units.pop(0)[1]()

            def drain_units(hp_limit):
                while proj_units and proj_units[0][0] <= hp_limit:
                    proj_units.pop(0)[1]()

            wq_t, wk_t = [], []
            for dc in range(NT):
                w = wpool.tile([128, D], F32R, tag="w")
                nc.sync.dma_start(w[:], wq_d[128 * dc:128 * (dc + 1), :].bitcast(F32R))
                wq_t.append(w)
            for dc in range(NT):
                w = wpool.tile([128, D], F32R, tag="w")
                nc.sync.dma_start(w[:], wk_d[128 * dc:128 * (dc + 1), :].bitcast(F32R))
                wk_t.append(w)

            for hp in range(NT):
                for sc in range(2):
                    proj_units.append((hp, (lambda nt=hp, sc=sc: emit_proj_unit(wq_t, nt, qT, sc))))
                    proj_units.append((hp, (lambda nt=hp, sc=sc: emit_proj_unit(wk_t, nt, kT, sc))))

            # ---- attention blocks ----
            for hp in range(NT):
                drain_units(hp)

                for qc in range(QC):
                    K = 4 * (qc + 1)
                    pts = {0: [], 1: []}
                    for kp in range(K // 2):
                        tiles = {}
                        for hh in range(2):
                            t_ = psb.tile([128, 1024], F32, tag="big", name=f"st_{hp}_{qc}_{kp}_{hh}")
                            tiles[hh] = t_
                        for j in range(2):
                            kc = 2 * kp + j
                            for hh in range(2):
                                rows = slice(64 * hh, 64 * (hh + 1))
                                nc.tensor.matmul(
                                    tiles[hh][:, 512 * j:512 * (j + 1)],
                                    kT[rows, hp, 128 * kc:128 * (kc + 1)],
                                    qT[rows, hp, 512 * qc:512 * (qc + 1)],
                                    start=True, stop=True,
                                    tile_position=(64 * hh, 0))
                        for hh in range(2):
                            pt = ptpool.tile([128, 1024], BF16, tag="pt")
                            nc.scalar.activation(pt[:], tiles[hh][:], AF.Exp)
                            halves = []
                            for j in range(2):
                                t = 2 * kp + j - 4 * qc
                                src_half = pt[:, 512 * j:512 * (j + 1)]
                                if 0 <= t <= 3:
                                    mh = mkpool.tile([128, 512], BF16, tag="mh")
                                    nc.vector.tensor_tensor(mh[:], src_half, msk[:, t, :], MULT)
                                    halves.append(mh[:])
                                else:
                                    halves.append(src_half)
                            pts[hh].append(halves)
                        pop_unit()

                    po_t = {}
                    for hh in range(2):
                        po_hh = pso.tile([65, 512], F32, tag="po", name=f"po_{hp}_{qc}_{hh}")
                        po_t[hh] = po_hh
                    for kc in range(K):
                        for hh in range(2):
                            h = 2 * hp + hh
                            nc.tensor.matmul(
                                po_t[hh][:],
                                vp[:, kc, 65 * h:65 * (h + 1)],
                                pts[hh][kc // 2][kc % 2],
                                start=(kc == 0), stop=(kc == K - 1))
                    for hh in range(2):
                        h = 2 * hp + hh
                        rows = slice(64 * hh, 64 * (hh + 1))
                        po = po_t[hh]
                        den = small.tile([1, 512], F32, tag="den")
                        nc.vector.tensor_copy(out=den[:], in_=po[64:65, :])
                        rc = small.tile([1, 512], F32, tag="rc")
                        nc.vector.reciprocal_approx_fast(out=rc[:], in_=den[:])
                        nc.sync.dma_start(recip_d[h, qc, :], rc[:])
                        rb = rbp.tile([64, 512], F32, tag="rb")
                        sl = recip_d[h, qc, :]
                        bc_ap = bass.AP(tensor=sl.tensor, offset=sl.offset,
                                        ap=[[0, 64]] + list(sl.ap))
                        nc.sync.dma_start(rb[:], bc_ap)
                        nc.vector.tensor_tensor(
                            outT[rows, hp, 512 * qc:512 * (qc + 1)],
                            po[0:64, :], rb[:], MULT)

            # ---- stage E: y = out @ W_proj ----
            wp_t = []
            for dc in range(NT):
                w = wpool.tile([128, D], F32R, tag="w")
                nc.sync.dma_start(w[:], wp_d[128 * dc:128 * (dc + 1), :].bitcast(F32R))
                wp_t.append(w)
            for st in range(ST):
                ps = psb.tile([128, 1024], F32, tag="big")
                for dc in range(NT):
                    nc.tensor.matmul(
                        ps[:, 0:512], outT[:, dc, 128 * st:128 * (st + 1)],
                        wp_t[dc][:, 0:512], start=(dc == 0), stop=(dc == NT - 1))
                for dc in range(NT):
                    nc.tensor.matmul(
                        ps[:, 512:768], outT[:, dc, 128 * st:128 * (st + 1)],
                        wp_t[dc][:, 512:768], start=(dc == 0), stop=(dc == NT - 1))
                y_sb = ypool.tile([128, D], F32, tag="y")
                nc.any.tensor_copy(out=y_sb[:], in_=ps[:, 0:768])
                nc.sync.dma_start(y_d[128 * st:128 * (st + 1), :], y_sb[:])

    nc.compile()
    return nc


def _get_compiled():
    global _compiled
    if _compiled is None:
        _compiled = _build_nc()
    return _compiled


def kernel(x, W_attn, W_proj):
    from concourse.bass_utils import run_bass_kernel_spmd

    x = np.asarray(x, dtype=np.float32)
    W_attn = np.asarray(W_attn, dtype=np.float32)
    W_proj = np.asarray(W_proj, dtype=np.float32)

    xT = np.ascontiguousarray(np.transpose(x, (0, 2, 1)))
    wq = np.ascontiguousarray(W_attn[:, 0:D]) * np.float32(0.125)
    wk = np.ascontiguousarray(W_attn[:, D:2 * D])
    wv = np.ascontiguousarray(W_attn[:, 2 * D:3 * D])
    masks = _build_masks()

    nc = _get_compiled()
    in_maps = [
        {"xT": xT[b], "wq": wq, "wk": wk, "wv": wv, "wp": W_proj, "masks": masks}
        for b in range(B)
    ]
    res = run_bass_kernel_spmd(nc, in_maps, list(range(NC_)))
    y = np.stack([res.results[b]["y"] for b in range(B)], axis=0)
    return y.astype(np.float32)


# revision 26
# speedup vs baseline: 1.2607x; 1.0218x over previous
"""Multi-head causal attention (B=8, S=1024, D=768, H=12) on 8 trn2 NeuronCores.

Strategy: data-parallel over batch (one batch element per core, no collectives).

Per-core dataflow (fp32r matmuls except A@V in bf16):
  - host passes x^T: Q^T/K^T via transposed projection (W stationary, x^T
    moving), V via natural projection (x^T stationary, W_v moving) -> no
    on-device transposes.
  - attention as S^T[k,q] = K @ Q^T per head; two heads (dh=64) packed into
    the 128-row PE array via row tiling, causal block-skip throughout.
  - softmax: exp on ScalarE straight out of PSUM ([128,1024] two-bank spans;
    1/8 scale folded into W_q host-side; no max-subtraction needed at these
    magnitudes); causal 0/1 bf16 mask multiply only on diagonal-crossing
    blocks; denominator free via a ones column appended to V (row 64 of the
    A@V PSUM); division folded into the PSUM->SBUF copy of A@V
    (fast reciprocal + DMA partition-broadcast through a DRAM scratch).
  - Q/K projection work-units are woven between attention tile groups so the
    PE fills exp-wait gaps and the HAM clock stays warm.
"""
import sys

if "/opt/trn_rl_repo" not in sys.path:
    sys.path.insert(0, "/opt/trn_rl_repo")

import numpy as np

B, S, D, H = 8, 1024, 768, 12
DH = 64
NC_ = 8
NT = D // 128    # 6
ST = S // 128    # 8
QC = S // 512    # 2
VPW = H * (DH + 1)  # 780

_compiled = None


def _build_masks():
    import ml_dtypes

    i = np.arange(128)[:, None, None]
    t = np.arange(4)[None, :, None]
    j = np.arange(512)[None, None, :]
    m = ((128 * t + i) <= j).astype(np.float32)
    return m.astype(ml_dtypes.bfloat16)


def _build_nc():
    import concourse.bass as bass
    import concourse.mybir as mybir
    import concourse.tile as tile
    from concourse import bacc

    F32 = mybir.dt.float32
    F32R = mybir.dt.float32r
    BF16 = mybir.dt.bfloat16
    AF = mybir.ActivationFunctionType
    MULT = mybir.AluOpType.mult

    nc = bacc.Bacc("TRN2", target_bir_lowering=False, debug=False)

    xT_d = nc.dram_tensor("xT", [D, S], F32, kind="ExternalInput")
    wq_d = nc.dram_tensor("wq", [D, D], F32, kind="ExternalInput")
    wk_d = nc.dram_tensor("wk", [D, D], F32, kind="ExternalInput")
    wv_d = nc.dram_tensor("wv", [D, D], F32, kind="ExternalInput")
    wp_d = nc.dram_tensor("wp", [D, D], F32, kind="ExternalInput")
    mask_d = nc.dram_tensor("masks", [128, 4, 512], BF16, kind="ExternalInput")
    y_d = nc.dram_tensor("y", [S, D], F32, kind="ExternalOutput")
    recip_d = nc.dram_tensor("recip_scratch", [H, QC, 512], F32)

    with tile.TileContext(nc) as tc:
        with (
            tc.tile_pool(name="static", bufs=1) as static,
            tc.tile_pool(name="w", bufs=12) as wpool,
            tc.tile_pool(name="pt", bufs=11) as ptpool,
            tc.tile_pool(name="small", bufs=2) as small,
            tc.tile_pool(name="rbp", bufs=2) as rbp,
            tc.tile_pool(name="mk", bufs=10) as mkpool,
            tc.tile_pool(name="y", bufs=2) as ypool,
            tc.tile_pool(name="psb", bufs=2, space="PSUM") as psb,
            tc.tile_pool(name="psproj", bufs=1, space="PSUM") as psproj,
            tc.tile_pool(name="pso", bufs=3, space="PSUM") as pso,
        ):
            # ---- persistent SBUF ----
            xT = static.tile([128, NT, S], F32R)
            qT = static.tile([128, NT, S], F32R)
            kT = static.tile([128, NT, S], F32R)
            vp = static.tile([128, ST, VPW], BF16)
            outT = static.tile([128, NT, S], F32R)
            msk = static.tile([128, 4, 512], BF16)

            for dc in range(NT):
                nc.sync.dma_start(xT[:, dc, :], xT_d[128 * dc:128 * (dc + 1), :].bitcast(F32R))
            nc.sync.dma_start(msk[:], mask_d[:])
            nc.vector.memset(vp[:], 1.0)

            # ---- stage C: v' = x @ W_v (natural layout) + ones cols ----
            wv_t = []
            for dc in range(NT):
                w = wpool.tile([128, D], F32R, tag="w")
                nc.sync.dma_start(w[:], wv_d[128 * dc:128 * (dc + 1), :].bitcast(F32R))
                wv_t.append(w)
            for st in range(ST):
                ps = psb.tile([128, 1024], F32, tag="big")
                for dc in range(NT):
                    nc.tensor.matmul(
                        ps[:, 0:512], xT[:, dc, 128 * st:128 * (st + 1)],
                        wv_t[dc][:, 0:512], start=(dc == 0), stop=(dc == NT - 1))
                for dc in range(NT):
                    nc.tensor.matmul(
                        ps[:, 512:768], xT[:, dc, 128 * st:128 * (st + 1)],
                        wv_t[dc][:, 512:768], start=(dc == 0), stop=(dc == NT - 1))
                dst = vp[:, st, :].rearrange("p (h e) -> p h e", e=DH + 1)
                nc.vector.tensor_copy(
                    out=dst[:, 0:8, 0:DH],
                    in_=ps[:, 0:512].rearrange("p (h d) -> p h d", d=DH))
                nc.vector.tensor_copy(
                    out=dst[:, 8:12, 0:DH],
                    in_=ps[:, 512:768].rearrange("p (h d) -> p h d", d=DH))

            # ---- Q^T/K^T projection work units (woven into attention) ----
            def emit_proj_unit(w_tiles, nt, dst, sc):
                ps = psproj.tile([128, 512], F32, tag="proj")
                for dc in range(NT):
                    nc.tensor.matmul(
                        ps[:],
                        w_tiles[dc][:, 128 * nt:128 * (nt + 1)],
                        xT[:, dc, 512 * sc:512 * (sc + 1)],
                        start=(dc == 0), stop=(dc == NT - 1))
                nc.any.tensor_copy(out=dst[:, nt, 512 * sc:512 * (sc + 1)], in_=ps[:])

            proj_units = []

            def pop_unit():
                if proj_units:
                    proj_units.pop(0)[1]()

            def drain_units(hp_limit):
                while proj_units and proj_units[0][0] <= hp_limit:
                    proj_units.pop(0)[1]()

            wq_t, wk_t = [], []
            for dc in range(NT):
                w = wpool.tile([128, D], F32R, tag="w")
                nc.sync.dma_start(w[:], wq_d[128 * dc:128 * (dc + 1), :].bitcast(F32R))
                wq_t.append(w)
            for dc in range(NT):
                w = wpool.tile([128, D], F32R, tag="w")
                nc.sync.dma_start(w[:], wk_d[128 * dc:128 * (dc + 1), :].bitcast(F32R))
                wk_t.append(w)

            for hp in range(NT):
                for sc in range(2):
                    proj_units.append((hp, (lambda nt=hp, sc=sc: emit_proj_unit(wq_t, nt, qT, sc))))
                    proj_units.append((hp, (lambda nt=hp, sc=sc: emit_proj_unit(wk_t, nt, kT, sc))))

            # ---- attention blocks ----
            for hp in range(NT):
                drain_units(hp)

                for qc in range(QC):
                    K = 4 * (qc + 1)
                    pts = {0: [], 1: []}
                    for kp in range(K // 2):
                        tiles = {}
                        for hh in range(2):
                            t_ = psb.tile([128, 1024], F32, tag="big", name=f"st_{hp}_{qc}_{kp}_{hh}")
                            tiles[hh] = t_
                        for j in range(2):
                            kc = 2 * kp + j
                            for hh in range(2):
                                rows = slice(64 * hh, 64 * (hh + 1))
                                nc.tensor.matmul(
                                    tiles[hh][:, 512 * j:512 * (j + 1)],
                                    kT[rows, hp, 128 * kc:128 * (kc + 1)],
                                    qT[rows, hp, 512 * qc:512 * (qc + 1)],
                                    start=True, stop=True,
                                    tile_position=(64 * hh, 0))
                        for hh in range(2):
                            pt = ptpool.tile([128, 1024], BF16, tag="pt")
                            nc.scalar.activation(pt[:], tiles[hh][:], AF.Exp)
                            halves = []
                            for j in range(2):
                                t = 2 * kp + j - 4 * qc
                                src_half = pt[:, 512 * j:512 * (j + 1)]
                                if 0 <= t <= 3:
                                    mh = mkpool.tile([128, 512], BF16, tag="mh")
                                    nc.vector.tensor_tensor(mh[:], src_half, msk[:, t, :], MULT)
                                    halves.append(mh[:])
                                else:
                                    halves.append(src_half)
                            pts[hh].append(halves)
                        pop_unit()

                    po_t = {}
                    for hh in range(2):
                        po_hh = pso.tile([65, 512], F32, tag="po", name=f"po_{hp}_{qc}_{hh}")
                        po_t[hh] = po_hh
                    for kc in range(K):
                        for hh in range(2):
                            h = 2 * hp + hh
                            nc.tensor.matmul(
                                po_t[hh][:],
                                vp[:, kc, 65 * h:65 * (h + 1)],
                                pts[hh][kc // 2][kc % 2],
                                start=(kc == 0), stop=(kc == K - 1))
                    for hh in range(2):
                        h = 2 * hp + hh
                        rows = slice(64 * hh, 64 * (hh + 1))
                        po = po_t[hh]
                        den = small.tile([1, 512], F32, tag="den")
                        nc.vector.tensor_copy(out=den[:], in_=po[64:65, :])
                        rc = small.tile([1, 512], F32, tag="rc")
                        nc.vector.reciprocal_approx_fast(out=rc[:], in_=den[:])
                        nc.sync.dma_start(recip_d[h, qc, :], rc[:])
                        rb = rbp.tile([64, 512], F32, tag="rb")
                        sl = recip_d[h, qc, :]
                        bc_ap = bass.AP(tensor=sl.tensor, offset=sl.offset,
                                        ap=[[0, 64]] + list(sl.ap))
                        nc.sync.dma_start(rb[:], bc_ap)
                        nc.vector.tensor_tensor(
                            outT[rows, hp, 512 * qc:512 * (qc + 1)],
                            po[0:64, :], rb[:], MULT)

            # ---- stage E: y = out @ W_proj ----
            wp_t = []
            for dc in range(NT):
                w = wpool.tile([128, D], F32R, tag="w")
                nc.sync.dma_start(w[:], wp_d[128 * dc:128 * (dc + 1), :].bitcast(F32R))
                wp_t.append(w)
            for st in range(ST):
                ps = psb.tile([128, 1024], F32, tag="big")
                for dc in range(NT):
                    nc.tensor.matmul(
                        ps[:, 0:512], outT[:, dc, 128 * st:128 * (st + 1)],
                        wp_t[dc][:, 0:512], start=(dc == 0), stop=(dc == NT - 1))
                for dc in range(NT):
                    nc.tensor.matmul(
                        ps[:, 512:768], outT[:, dc, 128 * st:128 * (st + 1)],
                        wp_t[dc][:, 512:768], start=(dc == 0), stop=(dc == NT - 1))
                y_sb = ypool.tile([128, D], F32, tag="y")
                nc.any.tensor_copy(out=y_sb[:], in_=ps[:, 0:768])
                nc.sync.dma_start(y_d[128 * st:128 * (st + 1), :], y_sb[:])

    nc.compile()
    return nc


def _get_compiled():
    global _compiled
    if _compiled is None:
        _compiled = _build_nc()
    return _compiled


def kernel(x, W_attn, W_proj):
    from concourse.bass_utils import run_bass_kernel_spmd

    x = np.asarray(x, dtype=np.float32)
    W_attn = np.asarray(W_attn, dtype=np.float32)
    W_proj = np.asarray(W_proj, dtype=np.float32)

    xT = np.ascontiguousarray(np.transpose(x, (0, 2, 1)))
    wq = np.ascontiguousarray(W_attn[:, 0:D]) * np.float32(0.125)
    wk = np.ascontiguousarray(W_attn[:, D:2 * D])
    wv = np.ascontiguousarray(W_attn[:, 2 * D:3 * D])
    masks = _build_masks()

    nc = _get_compiled()
    in_maps = [
        {"xT": xT[b], "wq": wq, "wk": wk, "wv": wv, "wp": W_proj, "masks": masks}
        for b in range(B)
    ]
    res = run_bass_kernel_spmd(nc, in_maps, list(range(NC_)))
    y = np.stack([res.results[b]["y"] for b in range(B)], axis=0)
    return y.astype(np.float32)
